# revision 16
# baseline (speedup 1.0000x reference)
"""Trainium2 8-core Bass kernel for the dense-transformer attention block.

Strategy: data-parallel over tokens. The 4096 tokens (2 batches x 2048 seq)
are split into 8 chunks of 512 (4 seq-chunks per batch, one per core).
Each core computes q/k/v projections + SwiGLU for its own 512 tokens with
all weights (streamed from HBM as bf16), all-gathers k/v within its 4-core
batch group, then computes full (mask-additive) attention for its 512
queries over all 2048 keys, and applies the output projection.

Layouts are feature-major ("transposed"): activations live as
[feature_partition, token_free] tiles so every matmul consumes operands
natively (lhsT = weight tile [K=128, M=128], rhs = activation [K=128, N=512]).
RoPE pairs are de-interleaved by permuting wq / wk_w2 columns on the host
(evens then odds per 128-dim head) so the rotation becomes contiguous
partition-block arithmetic; the host un-permutes the o_keys output.
1/sqrt(head_dim) is folded into wq on the host.

Softmax is computed un-normalized in the [ktok_partition, qtok_free]
direction (no transposes anywhere in attention):
  ST = kT_h^T-free matmul -> exp -> EP (bf16)
  denom = ones^T @ EP   (PE column-sum)
  outT_h = (v_h^T @ EP) * (1/denom broadcast)
"""

import sys

for _p in ("/opt/trn_rl_repo",):
    if _p not in sys.path:
        sys.path.insert(0, _p)

import numpy as np
import ml_dtypes

BF16 = ml_dtypes.bfloat16

DIM = 2048
N_HEADS = 16
N_KV = 8
HD = 128
HIDDEN = 8192
SEQ = 2048
BSZ = 2
CORES = 8
GROUP = 4            # cores per batch group
CHUNK = SEQ // GROUP  # 512 tokens per core
P = 128
TOK = CHUNK           # 512
KT_D = DIM // P       # 16 dim k-tiles
HT_N = HIDDEN // P    # 64 hidden tiles
RG = [[0, 1, 2, 3], [4, 5, 6, 7]]

# de-interleave permutation within a 128-dim head: evens then odds
PERM = np.concatenate([np.arange(0, HD, 2), np.arange(1, HD, 2)])
INV_PERM = np.argsort(PERM)


def _build_graph(has_mask: bool):
    import concourse.tile as tile
    from concourse import bacc, mybir

    f32 = mybir.dt.float32
    bf16 = mybir.dt.bfloat16
    AF = mybir.ActivationFunctionType

    nc = bacc.Bacc(None, target_bir_lowering=False)

    # ---- DRAM parameters (per-core shards / replicated weights) ----
    xt_p = nc.declare_dram_parameter("xt", [KT_D, P, TOK], bf16, isOutput=False)
    wq_p = nc.declare_dram_parameter("wqt", [N_HEADS, P, KT_D, P], bf16, isOutput=False)
    w1k_p = nc.declare_dram_parameter("w1k", [HT_N, P, KT_D, P], bf16, isOutput=False)
    w3k_p = nc.declare_dram_parameter("w3k", [HT_N, P, KT_D, P], bf16, isOutput=False)
    w1v_p = nc.declare_dram_parameter("w1v", [HT_N, P, KT_D, P], bf16, isOutput=False)
    w3v_p = nc.declare_dram_parameter("w3v", [HT_N, P, KT_D, P], bf16, isOutput=False)
    wk2_p = nc.declare_dram_parameter("wk2", [N_KV, P, HT_N, P], bf16, isOutput=False)
    wv2_p = nc.declare_dram_parameter("wv2", [HT_N, P, N_KV * HD], bf16, isOutput=False)
    wo_p = nc.declare_dram_parameter("wot", [KT_D, P, N_HEADS, P], bf16, isOutput=False)
    cos_p = nc.declare_dram_parameter("cost", [HD // 2, TOK], f32, isOutput=False)
    sin_p = nc.declare_dram_parameter("sint", [HD // 2, TOK], f32, isOutput=False)
    if has_mask:
        mask_p = nc.declare_dram_parameter("maskt", [SEQ // P, P, TOK], f32, isOutput=False)
    out_p = nc.declare_dram_parameter("outt", [KT_D, P, TOK], f32, isOutput=True)
    kout_p = nc.declare_dram_parameter("k_out", [N_KV, P, TOK], f32, isOutput=True)
    vout_p = nc.declare_dram_parameter("v_out", [TOK // P, P, N_KV * HD], f32, isOutput=True)

    with tile.TileContext(nc) as tc:
        # ---------- pools with explicit lifetimes ----------
        const = tc.alloc_tile_pool(name="const", bufs=1)
        dram = tc.alloc_tile_pool(name="dram", bufs=1, space="DRAM")
        xp = tc.alloc_tile_pool(name="xp", bufs=1)
        hp = tc.alloc_tile_pool(name="hp", bufs=1)

        ones_b = const.tile([P, 1], bf16)
        nc.any.memset(ones_b[:], 1.0)
        cosb = const.tile([HD // 2, TOK], f32)
        sinb = const.tile([HD // 2, TOK], f32)
        nc.sync.dma_start(cosb[:], cos_p[:])
        nc.sync.dma_start(sinb[:], sin_p[:])

        xtb = xp.tile([P, KT_D, TOK], bf16)
        nc.sync.dma_start(xtb[:], xt_p[:].rearrange("k p t -> p k t"))

        # AG bounce buffers (internal DRAM)
        k_ag_in = dram.tile([N_KV, P, TOK], bf16)
        k_ag_out = dram.tile([GROUP, N_KV, P, TOK], bf16)
        v_ag_in = dram.tile([TOK // P, P, N_KV * HD], bf16)
        v_ag_out = dram.tile([GROUP, TOK // P, P, N_KV * HD], bf16)

        def rope(psum_t, out_bf, kf=None):
            """psum_t [128, TOK] deinterleaved head -> rope -> out_bf bf16 tile.
            If kf given, also write f32 result there."""
            with tc.tile_pool(name="ropetmp", bufs=2) as rp:
                ta = rp.tile([HD // 2, TOK], f32, tag="ta")
                tb = rp.tile([HD // 2, TOK], f32, tag="tb")
                tc_ = rp.tile([HD // 2, TOK], f32, tag="tc")
                td = rp.tile([HD // 2, TOK], f32, tag="td")
                re, im = psum_t[0:64, :], psum_t[64:128, :]
                nc.vector.tensor_mul(ta[:], re, cosb[:])
                nc.vector.tensor_mul(tb[:], im, sinb[:])
                nc.vector.tensor_mul(tc_[:], re, sinb[:])
                nc.vector.tensor_mul(td[:], im, cosb[:])
                if kf is not None:
                    nc.vector.tensor_sub(kf[0:64, :], ta[:], tb[:])
                    nc.vector.tensor_add(kf[64:128, :], tc_[:], td[:])
                    nc.vector.tensor_copy(out_bf[0:64, :], kf[0:64, :])
                    nc.vector.tensor_copy(out_bf[64:128, :], kf[64:128, :])
                else:
                    nc.vector.tensor_sub(out_bf[0:64, :], ta[:], tb[:])
                    nc.vector.tensor_add(out_bf[64:128, :], tc_[:], td[:])

        def swiglu(w1_par, w3_par, hb):
            """hb[:, ht, :] = silu(w1^T x) * (w3^T x), bf16."""
            with (
                tc.tile_pool(name="wsw", bufs=3) as wp,
                tc.tile_pool(name="swps", bufs=4, space="PSUM") as pp,
                tc.tile_pool(name="swtmp", bufs=3) as sp,
            ):
                for ht in range(HT_N):
                    w1t = wp.tile([P, KT_D, P], bf16, tag="w1t")
                    w3t = wp.tile([P, KT_D, P], bf16, tag="w3t")
                    nc.sync.dma_start(w1t[:], w1_par[ht])
                    nc.sync.dma_start(w3t[:], w3_par[ht])
                    ps1 = pp.tile([P, TOK], f32, tag="ps1")
                    ps3 = pp.tile([P, TOK], f32, tag="ps3")
                    for kt in range(KT_D):
                        nc.tensor.matmul(ps1[:], w1t[:, kt, :], xtb[:, kt, :],
                                         start=(kt == 0), stop=(kt == KT_D - 1))
                    for kt in range(KT_D):
                        nc.tensor.matmul(ps3[:], w3t[:, kt, :], xtb[:, kt, :],
                                         start=(kt == 0), stop=(kt == KT_D - 1))
                    t1 = sp.tile([P, TOK], bf16, tag="silu")
                    nc.scalar.activation(t1[:], ps1[:], AF.Silu)
                    nc.vector.tensor_mul(hb[:, ht, :], ps3[:], t1[:])

        # ================= K branch =================
        if True:
            hb = hp.tile([P, HT_N, TOK], bf16, tag="hb")
            swiglu(w1k_p, w3k_p, hb)
            # k = w2^T @ h, feature-major kT [1024, TOK]; rope; AG-in + k_out
            with (
                tc.tile_pool(name="wk2p", bufs=2) as wp2,
                tc.tile_pool(name="kps", bufs=2, space="PSUM") as kpp,
                tc.tile_pool(name="kfp", bufs=2) as kfp,
            ):
                for m in range(N_KV):
                    w2t = wp2.tile([P, HT_N, P], bf16, tag="w2t")
                    nc.sync.dma_start(w2t[:], wk2_p[m])
                    psk = kpp.tile([P, TOK], f32, tag="psk")
                    for ht in range(HT_N):
                        nc.tensor.matmul(psk[:], w2t[:, ht, :], hb[:, ht, :],
                                         start=(ht == 0), stop=(ht == HT_N - 1))
                    kf = kfp.tile([P, TOK], f32, tag="kf")
                    kab = kfp.tile([P, TOK], bf16, tag="kab")
                    rope(psk, kab, kf=kf)
                    nc.sync.dma_start(kout_p[m], kf[:])
                    nc.sync.dma_start(k_ag_in[m], kab[:])
            nc.gpsimd.collective_compute(
                "AllGather", mybir.AluOpType.bypass,
                ins=[k_ag_in.opt()], outs=[k_ag_out.opt()], replica_groups=RG,
            )

            # ================= V branch =================
            hb2 = hp.tile([P, HT_N, TOK], bf16, tag="hb")
            swiglu(w1v_p, w3v_p, hb2)
            # v = h^T-tiles @ wv2, token-major [TOK, 1024]
            with (
                tc.tile_pool(name="wv2p", bufs=4) as wvp,
                tc.tile_pool(name="vps", bufs=1, space="PSUM") as vpp,
                tc.tile_pool(name="vfp", bufs=4) as vfp,
            ):
                psv = [[vpp.tile([P, TOK], f32, tag=f"psv{tt}_{n}",
                                 name=f"psv{tt}_{n}")
                        for n in range(2)] for tt in range(TOK // P)]
                for ht in range(HT_N):
                    wvt = wvp.tile([P, N_KV * HD], bf16, tag="wvt")
                    nc.sync.dma_start(wvt[:], wv2_p[ht])
                    for tt in range(TOK // P):
                        for n in range(2):
                            nc.tensor.matmul(
                                psv[tt][n][:],
                                hb2[:, ht, tt * P:(tt + 1) * P],
                                wvt[:, n * TOK:(n + 1) * TOK],
                                start=(ht == 0), stop=(ht == HT_N - 1))
                for tt in range(TOK // P):
                    for n in range(2):
                        vf = vfp.tile([P, TOK], f32, tag="vf")
                        vab = vfp.tile([P, TOK], bf16, tag="vab")
                        nc.vector.tensor_copy(vf[:], psv[tt][n][:])
                        nc.vector.tensor_copy(vab[:], vf[:])
                        nc.sync.dma_start(vout_p[tt][:, n * TOK:(n + 1) * TOK], vf[:])
                        nc.sync.dma_start(v_ag_in[tt][:, n * TOK:(n + 1) * TOK], vab[:])
            nc.gpsimd.collective_compute(
                "AllGather", mybir.AluOpType.bypass,
                ins=[v_ag_in.opt()], outs=[v_ag_out.opt()], replica_groups=RG,
            )

        # ================= Q projection =================
        hp.release()
        qp = tc.alloc_tile_pool(name="qp", bufs=1, side="right")
        qtb = qp.tile([P, N_HEADS, TOK], bf16)            # roped q, deint
        with (
            tc.tile_pool(name="wqp", bufs=3) as wqp,
            tc.tile_pool(name="qps", bufs=2, space="PSUM") as qpp,
        ):
            for h in range(N_HEADS):
                wqt = wqp.tile([P, KT_D, P], bf16, tag="wqt")
                nc.sync.dma_start(wqt[:], wq_p[h])
                psq = qpp.tile([P, TOK], f32, tag="psq")
                for kt in range(KT_D):
                    nc.tensor.matmul(psq[:], wqt[:, kt, :], xtb[:, kt, :],
                                     start=(kt == 0), stop=(kt == KT_D - 1))
                rope(psq, qtb[:, h, :])

        # load gathered k/v
        xp.release()
        aop = tc.alloc_tile_pool(name="aop", bufs=1)
        kvp = tc.alloc_tile_pool(name="kvp", bufs=1)
        ktb = kvp.tile([P, N_KV, GROUP, TOK], bf16)       # gathered roped kT
        vb = kvp.tile([P, SEQ // P, N_KV * HD], bf16)     # gathered token-major v
        aob = aop.tile([P, N_HEADS, TOK], bf16)           # attention out (v-dims)
        for h2 in range(N_KV):
            nc.sync.dma_start(ktb[:, h2], k_ag_out[:, h2].rearrange("r p t -> p r t"))
        nc.sync.dma_start(vb[:], v_ag_out[:].rearrange("r s p n -> p (r s) n"))
        if has_mask:
            maskb = kvp.tile([P, SEQ // P, TOK], f32, tag="maskb")
            nc.sync.dma_start(maskb[:], mask_p[:].rearrange("k p t -> p k t"))

        # ================= Attention =================
        with (
            tc.tile_pool(name="eppool", bufs=2) as epp,
            tc.tile_pool(name="stps", bufs=3, space="PSUM") as stp,
            tc.tile_pool(name="pvps", bufs=2, space="PSUM") as pvp,
            tc.tile_pool(name="dnps", bufs=2, space="PSUM") as dnp,
            tc.tile_pool(name="dnsb", bufs=2) as dns,
        ):
            KTILES = SEQ // P  # 16
            for h in range(N_HEADS):
                h2 = h // 2
                ep = epp.tile([P, KTILES, TOK], bf16, tag="ep")
                for kt in range(KTILES):
                    r, sub = kt // GROUP, kt % GROUP
                    st = stp.tile([P, TOK], f32, tag="st")
                    nc.tensor.matmul(st[:],
                                     ktb[:, h2, r, sub * P:(sub + 1) * P],
                                     qtb[:, h, :], start=True, stop=True)
                    if has_mask:
                        nc.vector.tensor_add(st[:], st[:], maskb[:, kt, :])
                    nc.scalar.activation(ep[:, kt, :], st[:], AF.Exp)
                dn = dnp.tile([1, TOK], f32, tag="dn")
                for kt in range(KTILES):
                    nc.tensor.matmul(dn[:], ones_b[:], ep[:, kt, :],
                                     start=(kt == 0), stop=(kt == KTILES - 1))
                pv = pvp.tile([P, TOK], f32, tag="pv")
                for kt in range(KTILES):
                    nc.tensor.matmul(pv[:], vb[:, kt, h2 * HD:(h2 + 1) * HD],
                                     ep[:, kt, :],
                                     start=(kt == 0), stop=(kt == KTILES - 1))
                rd = dns.tile([1, TOK], f32, tag="rd")
                rdb = dns.tile([P, TOK], f32, tag="rdb")
                nc.vector.reciprocal(rd[:], dn[:])
                nc.gpsimd.partition_broadcast(rdb[:], rd[:])
                nc.vector.tensor_mul(aob[:, h, :], pv[:], rdb[:])

        # ================= Output projection =================
        kvp.release()
        qp.release()
        with (
            tc.tile_pool(name="wop", bufs=3) as wop,
            tc.tile_pool(name="ops", bufs=2, space="PSUM") as opp,
            tc.tile_pool(name="osb", bufs=3) as osb,
        ):
            for m in range(KT_D):
                wot = wop.tile([P, N_HEADS, P], bf16, tag="wot")
                nc.sync.dma_start(wot[:], wo_p[m])
                pso = opp.tile([P, TOK], f32, tag="pso")
                for kt in range(N_HEADS):
                    nc.tensor.matmul(pso[:], wot[:, kt, :], aob[:, kt, :],
                                     start=(kt == 0), stop=(kt == N_HEADS - 1))
                of = osb.tile([P, TOK], f32, tag="of")
                nc.vector.tensor_copy(of[:], pso[:])
                nc.sync.dma_start(out_p[m], of[:])
        aop.release()
        dram.release()
        const.release()
    nc.compile()
    return nc


_GRAPH_CACHE = {}


def _get_graph(has_mask: bool):
    if has_mask not in _GRAPH_CACHE:
        _GRAPH_CACHE[has_mask] = _build_graph(has_mask)
    return _GRAPH_CACHE[has_mask]


def _host_prep(inputs):
    x = np.ascontiguousarray(np.asarray(inputs["x"], np.float32))
    wq = np.asarray(inputs["wq"], np.float32)
    wk1 = np.asarray(inputs["wk_w1"], np.float32)
    wk2 = np.asarray(inputs["wk_w2"], np.float32)
    wk3 = np.asarray(inputs["wk_w3"], np.float32)
    wv1 = np.asarray(inputs["wv_w1"], np.float32)
    wv2 = np.asarray(inputs["wv_w2"], np.float32)
    wv3 = np.asarray(inputs["wv_w3"], np.float32)
    wo = np.asarray(inputs["wo"], np.float32)
    cos = np.asarray(inputs["freqs_cos"], np.float32)
    sin = np.asarray(inputs["freqs_sin"], np.float32)
    mask = np.asarray(inputs["mask"], np.float32)
    has_mask = bool(np.any(mask))

    # permute rope-pair columns (evens then odds per head), fold 1/sqrt(HD) into wq
    wq_p = (wq.reshape(DIM, N_HEADS, HD)[:, :, PERM] / np.float32(np.sqrt(HD))
            ).reshape(DIM, DIM)
    wk2_p = wk2.reshape(HIDDEN, N_KV, HD)[:, :, PERM].reshape(HIDDEN, N_KV * HD)

    def tile4(w, nin, nout):  # [nin*P, nout*P] -> [nout, P, nin, P]
        return np.ascontiguousarray(
            w.reshape(nin, P, nout, P).transpose(2, 1, 0, 3).astype(BF16))

    shared = {
        "wqt": tile4(wq_p, KT_D, N_HEADS),
        "w1k": tile4(wk1, KT_D, HT_N),
        "w3k": tile4(wk3, KT_D, HT_N),
        "w1v": tile4(wv1, KT_D, HT_N),
        "w3v": tile4(wv3, KT_D, HT_N),
        "wk2": tile4(wk2_p, HT_N, N_KV),
        "wv2": np.ascontiguousarray(wv2.reshape(HT_N, P, N_KV * HD).astype(BF16)),
        "wot": tile4(wo, KT_D, KT_D),
    }
    cosT = np.ascontiguousarray(cos.T)  # [64, SEQ]
    sinT = np.ascontiguousarray(sin.T)

    in_maps = []
    for c in range(CORES):
        b, ch = c // GROUP, c % GROUP
        rows = slice(ch * CHUNK, (ch + 1) * CHUNK)
        m = dict(shared)
        m["xt"] = np.ascontiguousarray(
            x[b, rows].T.reshape(KT_D, P, TOK).astype(BF16))
        m["cost"] = np.ascontiguousarray(cosT[:, rows])
        m["sint"] = np.ascontiguousarray(sinT[:, rows])
        if has_mask:
            m["maskt"] = np.ascontiguousarray(
                mask[0, 0, rows, :].T.reshape(SEQ // P, P, TOK))
        in_maps.append(m)
    return in_maps, has_mask


def _assemble(results):
    out = np.empty((BSZ, SEQ, DIM), np.float32)
    o_keys = np.empty((BSZ, SEQ, N_KV, HD), np.float32)
    o_vals = np.empty((BSZ, SEQ, N_KV, HD), np.float32)
    for c in range(CORES):
        b, ch = c // GROUP, c % GROUP
        rows = slice(ch * CHUNK, (ch + 1) * CHUNK)
        r = results[c]
        out[b, rows] = np.asarray(r["outt"], np.float32).reshape(DIM, TOK).T
        k = np.asarray(r["k_out"], np.float32)          # [8, 128, TOK] deint
        k = k[:, INV_PERM, :]                            # undo perm on hd axis
        o_keys[b, rows] = k.transpose(2, 0, 1)
        o_vals[b, rows] = np.asarray(r["v_out"], np.float32).reshape(
            TOK, N_KV, HD)
    return out, o_keys, o_vals


def run_on_cores(inputs, trace=False, **kw):
    from concourse.bass_utils import run_bass_kernel_spmd
    in_maps, has_mask = _host_prep(inputs)
    nc = _get_graph(has_mask)
    res = run_bass_kernel_spmd(nc, in_maps, list(range(CORES)), trace=trace, **kw)
    return _assemble(res.results), res


def kernel(**inputs):
    (out, o_keys, o_vals), _ = run_on_cores(inputs)
    return out, o_keys, o_vals
